# revision 1
# baseline (speedup 1.0000x reference)
"""GNN message-passing (SpMM + mean-normalize + bias) Trainium2 kernel.

out[r] = (sum_{e: rows[e]==r} vals[e] * x[cols[e]]) / deg[r] + bias,
deg[r] = sum vals[e], rows with deg==0 -> bias.

Strategy (8 NeuronCores, SPMD):
  - Pad N=40000 rows to 40960 = 320 bins x 128 rows. Core c owns bins
    [40c, 40c+40) => output rows [5120c, 5120(c+1)).  Edges are bucketed by
    destination bin on the host (this is the sharding step), so no
    cross-core collectives are needed.
  - Per bin, edges are split into a low group (col < 32768) and a high
    group (col >= 32768), each padded to a multiple of 128 with null
    edges (val=0), because dma_gather carries int16 indices.
  - Device per bin: two dma_gather ops fetch x rows for all edges
    (512B/row), slot i <- (partition i%128, chunk i//128).  For each
    128-edge chunk a one-hot selection matrix S[t,r] = (ri[t]==r)*val[t]
    is built on the vector engine from an iota tile, then the tensor
    engine computes psum[r,f] += S^T @ xg (PSUM accumulation) and
    deg[r] += S^T @ ones.  Epilogue normalizes by deg (deg==0 -> 0),
    adds bias, and DMAs the 128-row block out.
"""
import sys

sys.path.insert(0, "/opt/trn_rl_repo")

import numpy as np

N_NODES = 40000
N_EDGES = 640000
D = 128
P = 128
N_CORES = 8
BINS_PER_CORE = 40
N_BINS = N_CORES * BINS_PER_CORE          # 320 (rows padded to 40960)
SPLIT = 32768                             # int16-safe index split

_plan_cache: dict = {}


def _patch_ldw_opt():
    """Enable walrus's LDW dedup pass (second matmul on the same stationary
    S skips its LoadWeights)."""
    import concourse.bass_utils as bu

    if getattr(bu, "_ldw_patched", False):
        return
    orig = bu.run_command

    def patched(argv, **kw):
        argv = ["--enable-ldw-opt=true" if a == "--enable-ldw-opt=false" else a
                for a in argv]
        return orig(argv, **kw)

    bu.run_command = patched
    bu._ldw_patched = True


def _build_program(NLO, NHI, NXL, NXH):
    """Build+compile the SPMD Bass program for the given per-bin-position
    chunk schedule (shared by all cores)."""
    import concourse.bacc as bacc
    import concourse.bass as bass
    import concourse.tile as tile
    from concourse import mybir

    _patch_ldw_opt()

    NCH = [NLO[p] + NHI[p] for p in range(BINS_PER_CORE)]
    F = sum(NCH)
    F16 = F * 8

    NQ = 4
    nc = bacc.Bacc(num_swdge_queues=NQ)
    x_d = nc.dram_tensor("x", [N_NODES, D], mybir.dt.float32, kind="ExternalInput")
    idx_d = nc.dram_tensor("idx", [P, F16], mybir.dt.int16, kind="ExternalInput")
    meta_d = nc.dram_tensor("meta", [P, P + 4 * F + BINS_PER_CORE],
                            mybir.dt.float32, kind="ExternalInput")
    bias_d = nc.dram_tensor("bias", [P, D], mybir.dt.float32, kind="ExternalInput")
    degrow_d = nc.dram_tensor("degrow", [1, BINS_PER_CORE * P], mybir.dt.float32,
                              kind="ExternalInput")
    biasrow_d = nc.dram_tensor("biasrow", [1, D], mybir.dt.float32,
                               kind="ExternalInput")
    out_d = nc.dram_tensor("out", [BINS_PER_CORE * P, D], mybir.dt.float32,
                           kind="ExternalOutput")

    with tile.TileContext(nc) as tc:
        with tc.tile_pool(name="persist", bufs=1) as persist, \
             tc.tile_pool(name="xgp", bufs=5) as xgp, \
             tc.tile_pool(name="spool", bufs=12) as spool, \
             tc.tile_pool(name="outp", bufs=3) as outp, \
             tc.tile_pool(name="ep", bufs=2) as ep, \
             tc.tile_pool(name="actp", bufs=3) as actp, \
             tc.tile_pool(name="ps", bufs=4, space="PSUM") as ps, \
             tc.tile_pool(name="psd", bufs=2, space="PSUM") as psd:
            idx_t = persist.tile([P, F16], mybir.dt.int16)
            meta_t = persist.tile([P, P + 4 * F + BINS_PER_CORE],
                                  mybir.dt.float32)
            bias_t = persist.tile([P, D], mybir.dt.float32)
            degrow_t = persist.tile([1, BINS_PER_CORE * P], mybir.dt.float32)
            biasrow_t = persist.tile([1, D], mybir.dt.float32)
            ones_t = persist.tile([P, 1], mybir.dt.float32)
            nc.sync.dma_start(out=idx_t[:], in_=idx_d[:, :])
            nc.sync.dma_start(out=meta_t[:], in_=meta_d[:, :])
            nc.sync.dma_start(out=bias_t[:], in_=bias_d[:, :])
            nc.sync.dma_start(out=degrow_t[:], in_=degrow_d[:, :])
            nc.sync.dma_start(out=biasrow_t[:], in_=biasrow_d[:, :])
            nc.vector.memset(ones_t[:], 1.0)
            iota_t = meta_t[:, 0:P]

            maxch = max(NCH)
            for _w in range(5):
                wt = xgp.tile([P, maxch * D], mybir.dt.float32, tag="xg")
                nc.vector.memset(wt[:], 0.0)
            # dma_gather is limited to 1024 indices (8 chunks) per call
            GMAX = 8
            _gq = [0]
            for b in range(BINS_PER_CORE):
                offb = sum(NCH[:b])
                nch, nlo, nhi = NCH[b], NLO[b], NHI[b]
                xg = xgp.tile([P, nch * D], mybir.dt.float32, tag="xg")
                subs = []  # (chunk off, n chunks, is_high, exact idx count)
                for s in range(0, nlo, GMAX):
                    n = min(GMAX, nlo - s)
                    nidx = max(16, min(n * 128, NXL[b] - s * 128))
                    subs.append((s, n, False, nidx))
                for s in range(0, nhi, GMAX):
                    n = min(GMAX, nhi - s)
                    nidx = max(16, min(n * 128, NXH[b] - s * 128))
                    subs.append((nlo + s, n, True, nidx))
                for s, n, hi, nidx in subs:
                    nc.gpsimd.dma_gather(
                        out_ap=xg[:, s * D : (s + n) * D].rearrange(
                            "p (k w) -> p k w", k=n),
                        in_ap=(x_d[SPLIT:N_NODES, :] if hi else x_d[0:SPLIT, :]),
                        idxs_ap=idx_t[:, (offb + s) * 8 : (offb + s + n) * 8],
                        num_idxs=nidx,
                        num_idxs_reg=nidx,
                        elem_size=D,
                        queue_num=_gq[0] % NQ,
                    )
                    _gq[0] += 1
                psum = ps.tile([P, D], mybir.dt.float32, tag="psum")
                nc.tensor.matmul(out=psum[:],
                                 lhsT=degrow_t[:, b * P : (b + 1) * P],
                                 rhs=biasrow_t[:, :],
                                 start=True, stop=False)
                # tiny PE reads of xg: absorb the gather-DMA semaphore waits
                # so real matmuls carry only the DVE wait
                dummy = psd.tile([1, 1], mybir.dt.float32, tag="dummy")
                for s, n, hi, nidx in subs:
                    nc.tensor.matmul(out=dummy[:], lhsT=xg[:1, s * D : s * D + 1],
                                     rhs=xg[:1, s * D : s * D + 1],
                                     start=True, stop=True)
                NR0 = P + 2 * F + BINS_PER_CORE          # negri block offset
                NV0 = NR0 + F                             # negval block offset
                for c in range(nch):
                    S = spool.tile([P, P], mybir.dt.float32, tag="S")
                    if False:  # ACT S-build: correct but slower (act-table thrash)
                        # ACT path: S = Relu(val - val*(iota-ri)^2)
                        p1 = actp.tile([P, P], mybir.dt.float32, tag="p1")
                        nc.scalar.activation(
                            out=p1[:], in_=iota_t,
                            func=mybir.ActivationFunctionType.Square,
                            bias=meta_t[:, NR0 + offb + c : NR0 + offb + c + 1],
                            scale=1.0)
                        nc.scalar.activation(
                            out=S[:], in_=p1[:],
                            func=mybir.ActivationFunctionType.Relu,
                            bias=meta_t[:, P + F + offb + c : P + F + offb + c + 1],
                            scale=meta_t[:, NV0 + offb + c : NV0 + offb + c + 1])
                    elif False:  # gpsimd S-build: ~2.4us/op, too slow
                        nc.gpsimd.tensor_scalar(
                            out=S[:], in0=iota_t,
                            scalar1=meta_t[:, P + offb + c : P + offb + c + 1],
                            scalar2=meta_t[:, P + F + offb + c : P + F + offb + c + 1],
                            op0=mybir.AluOpType.is_equal, op1=mybir.AluOpType.mult,
                        )
                    else:
                        nc.vector.tensor_scalar(
                            out=S[:], in0=iota_t,
                            scalar1=meta_t[:, P + offb + c : P + offb + c + 1],
                            scalar2=meta_t[:, P + F + offb + c : P + F + offb + c + 1],
                            op0=mybir.AluOpType.is_equal, op1=mybir.AluOpType.mult,
                        )
                    nc.tensor.matmul(out=psum[:], lhsT=S[:],
                                     rhs=xg[:, c * D : (c + 1) * D],
                                     start=False, stop=(c == nch - 1))
                # epilogue: out = (agg + deg*bias) * rdeg  (on ACT)
                o_t = outp.tile([P, D], mybir.dt.float32, tag="o")
                nc.scalar.activation(
                    out=o_t[:], in_=psum[:],
                    func=mybir.ActivationFunctionType.Copy,
                    scale=meta_t[:, P + 2 * F + b : P + 2 * F + b + 1])
                nc.sync.dma_start(out=out_d[b * P : (b + 1) * P, :], in_=o_t[:])

    nc.compile()
    return nc


def _cdiv(a, b):
    return -(-a // b)


def _preprocess(x, edge_rows, edge_cols, adj_vals, bias):
    """Bucket edges by destination bin, split low/high cols, pad, and build
    per-core device input arrays."""
    bin_id = (edge_rows // P).astype(np.int64)
    is_high = (edge_cols >= SPLIT).astype(np.int64)
    order = np.lexsort((is_high, bin_id))
    b_s = bin_id[order]
    h_s = is_high[order]
    col_s = edge_cols[order].astype(np.int32)
    val_s = adj_vals[order].astype(np.float32)
    ri_s = (edge_rows[order] - b_s * P).astype(np.float32)

    n_tot = np.bincount(b_s, minlength=N_BINS)
    n_hi = np.bincount(b_s, weights=h_s, minlength=N_BINS).astype(np.int64)
    n_lo = n_tot - n_hi
    starts = np.concatenate([[0], np.cumsum(n_tot)])[:N_BINS]

    # per-position chunk counts, shared across cores (SPMD)
    NLO = [max(1, int(max(_cdiv(int(n_lo[40 * c + p]), P)
                          for c in range(N_CORES))))
           for p in range(BINS_PER_CORE)]
    NHI = [max(1, int(max(_cdiv(int(n_hi[40 * c + p]), P)
                          for c in range(N_CORES))))
           for p in range(BINS_PER_CORE)]
    NCH = [NLO[p] + NHI[p] for p in range(BINS_PER_CORE)]
    F = sum(NCH)
    NXL = [max(16, 16 * int(_cdiv(int(max(n_lo[40 * c + p] for c in range(N_CORES))), 16)))
           for p in range(BINS_PER_CORE)]
    NXH = [max(16, 16 * int(_cdiv(int(max(n_hi[40 * c + p] for c in range(N_CORES))), 16)))
           for p in range(BINS_PER_CORE)]

    iota_np = np.tile(np.arange(P, dtype=np.float32), (P, 1))
    bias_rep = np.tile(np.asarray(bias, dtype=np.float32), (P, 1))
    deg = np.bincount(edge_rows, weights=adj_vals.astype(np.float64),
                      minlength=N_BINS * P).astype(np.float32)
    rdeg = np.ones(N_BINS * P, np.float32)
    nz = deg != 0
    rdeg[nz] = (1.0 / deg[nz]).astype(np.float32)
    deg = deg.copy()
    deg[~nz] = 1.0

    in_maps = []
    for c in range(N_CORES):
        idx_parts = []
        ri_arr = np.zeros((P, F), np.float32)
        val_arr = np.zeros((P, F), np.float32)
        off = 0
        for p in range(BINS_PER_CORE):
            g = 40 * c + p
            s = int(starts[g])
            nl, nh = int(n_lo[g]), int(n_hi[g])
            lo_pad, hi_pad = NLO[p] * P, NHI[p] * P
            cols_lo = np.zeros(lo_pad, np.int32)
            cols_lo[:nl] = col_s[s : s + nl]
            cols_hi = np.full(hi_pad, SPLIT, np.int32)
            cols_hi[:nh] = col_s[s + nl : s + nl + nh]
            ris = np.zeros(lo_pad + hi_pad, np.float32)
            ris[:nl] = ri_s[s : s + nl]
            ris[lo_pad : lo_pad + nh] = ri_s[s + nl : s + nl + nh]
            vals = np.zeros(lo_pad + hi_pad, np.float32)
            vals[:nl] = val_s[s : s + nl]
            vals[lo_pad : lo_pad + nh] = val_s[s + nl : s + nl + nh]
            # wrapped int16 idx layout: idx i at [i%16, i//16], replicated 8x
            wlo = cols_lo.reshape(-1, 16).T.astype(np.int16)
            whi = (cols_hi - SPLIT).reshape(-1, 16).T.astype(np.int16)
            idx_parts.append(np.tile(wlo, (8, 1)))
            idx_parts.append(np.tile(whi, (8, 1)))
            nch = NCH[p]
            ri_arr[:, off : off + nch] = ris.reshape(nch, P).T
            val_arr[:, off : off + nch] = vals.reshape(nch, P).T
            off += nch
        idx_np = np.concatenate(idx_parts, axis=1)
        rdeg_arr = rdeg[5120 * c : 5120 * (c + 1)].reshape(BINS_PER_CORE, P).T
        meta_np = np.concatenate([iota_np, ri_arr, val_arr,
                                  np.ascontiguousarray(rdeg_arr),
                                  -ri_arr, -val_arr], axis=1)
        in_maps.append({
            "x": np.ascontiguousarray(x, dtype=np.float32),
            "idx": idx_np,
            "meta": meta_np,
            "bias": bias_rep,
            "degrow": np.ascontiguousarray(
                deg[5120 * c : 5120 * (c + 1)].reshape(1, -1)),
            "biasrow": np.asarray(bias, np.float32).reshape(1, -1),
        })
    return tuple(NLO), tuple(NHI), tuple(NXL), tuple(NXH), in_maps


def _run(x, edge_rows, edge_cols, adj_vals, bias, trace=False, trace_cores=None):
    from concourse.bass_utils import run_bass_kernel_spmd

    NLO, NHI, NXL, NXH, in_maps = _preprocess(x, edge_rows, edge_cols,
                                              adj_vals, bias)
    key = (NLO, NHI, NXL, NXH)
    if key not in _plan_cache:
        _plan_cache[key] = _build_program(list(NLO), list(NHI), list(NXL),
                                          list(NXH))
    nc = _plan_cache[key]
    kw = {}
    if trace:
        kw["trace"] = True
        if trace_cores is not None:
            kw["trace_cores"] = trace_cores
    res = run_bass_kernel_spmd(nc, in_maps, core_ids=list(range(N_CORES)), **kw)
    out = np.concatenate([res.results[c]["out"] for c in range(N_CORES)], axis=0)
    return out[:N_NODES].astype(np.float32), res


def kernel(x, edge_rows, edge_cols, adj_vals, bias):
    out, _ = _run(np.asarray(x), np.asarray(edge_rows), np.asarray(edge_cols),
                  np.asarray(adj_vals), np.asarray(bias))
    return out



# revision 10
# speedup vs baseline: 1.3011x; 1.3011x over previous
"""GNN message-passing (SpMM + mean-normalize + bias) Trainium2 kernel.

out[r] = (sum_{e: rows[e]==r} vals[e] * x[cols[e]]) / deg[r] + bias,
deg[r] = sum vals[e], rows with deg==0 -> bias.

Strategy (8 NeuronCores, SPMD):
  - Pad N=40000 rows to 40960 = 320 bins x 128 rows. Bins are assigned to
    (core, position) pairs in balanced order (sorted by edge count, rank
    groups of 8 across cores) so the SPMD-shared chunk schedule has minimal
    padding. Host unscrambles the output rows.
  - Per bin, edges are split into a low group (col < 32768) and a high
    group (col >= 32768), each padded to a multiple of 128 with null
    edges (val=0), because dma_gather carries int16 indices. Edges are
    sorted by col inside each group for DMA locality.
  - x is converted to fp16 on the host. Gathers fetch 256B rows; bins are
    processed in groups of 5 sharing one xg tile so gather calls batch up
    to 1024 indices regardless of bin boundaries.
  - Device per chunk of 128 edges: a one-hot selection matrix
    S[t,r] = (ri[t]==r)*val[t] (fp16) is built on the vector engine from an
    iota tile, then the tensor engine computes psum[r,f] += S^T @ xg
    (PSUM fp32 accumulation). The accumulation is seeded with
    deg[r]*bias[f] via a tiny fp16 matmul, so the epilogue is just
    out = psum * rdeg on the scalar engine (deg==0 rows have rdeg=1,
    deg=1 -> out=bias).
"""
import sys

sys.path.insert(0, "/opt/trn_rl_repo")

import numpy as np

N_NODES = 40000
N_EDGES = 640000
D = 128
P = 128
N_CORES = 8
BINS_PER_CORE = 40
N_BINS = N_CORES * BINS_PER_CORE          # 320 (rows padded to 40960)
SPLIT = 32768                             # int16-safe index split
GROUP = 5                                 # bins per gather group
GMAX = 8                                  # chunks (1024 idx) per dma_gather

_plan_cache: dict = {}


def _patch_ldw_opt():
    """Enable walrus's LDW dedup pass (second matmul on the same stationary
    S skips its LoadWeights)."""
    import concourse.bass_utils as bu

    if getattr(bu, "_ldw_patched", False):
        return
    orig = bu.run_command

    def patched(argv, **kw):
        argv = ["--enable-ldw-opt=true" if a == "--enable-ldw-opt=false" else a
                for a in argv]
        return orig(argv, **kw)

    bu.run_command = patched
    bu._ldw_patched = True


def _cdiv(a, b):
    return -(-a // b)


def _build_program(NLO, NHI):
    """Build+compile the SPMD Bass program for the given per-position chunk
    schedule (shared by all cores)."""
    import concourse.bacc as bacc
    import concourse.tile as tile
    from concourse import mybir

    NCH = [NLO[p] + NHI[p] for p in range(BINS_PER_CORE)]
    F = sum(NCH)

    NQ = 4
    nc = bacc.Bacc(num_swdge_queues=NQ)
    x_d = nc.dram_tensor("x", [N_NODES, D], mybir.dt.float16,
                         kind="ExternalInput")
    idx_d = nc.dram_tensor("idx", [P, F * 8], mybir.dt.int16,
                           kind="ExternalInput")
    iota_d = nc.dram_tensor("iota", [P, P], mybir.dt.float16,
                            kind="ExternalInput")
    meta_d = nc.dram_tensor("meta", [P, 2 * F + BINS_PER_CORE],
                            mybir.dt.float32, kind="ExternalInput")
    degrow_d = nc.dram_tensor("degrow", [1, BINS_PER_CORE * P],
                              mybir.dt.float16, kind="ExternalInput")
    biasrow_d = nc.dram_tensor("biasrow", [1, D], mybir.dt.float16,
                               kind="ExternalInput")
    out_d = nc.dram_tensor("out", [BINS_PER_CORE * P, D], mybir.dt.float32,
                           kind="ExternalOutput")

    # group schedule: positions [5g, 5g+5); chunk layout per group:
    # [lo p0][lo p1]..[lo p4][hi p0]..[hi p4]
    groups = []
    for g in range(BINS_PER_CORE // GROUP):
        pos = list(range(GROUP * g, GROUP * (g + 1)))
        glo = sum(NLO[p] for p in pos)
        ghi = sum(NHI[p] for p in pos)
        groups.append((pos, glo, ghi))
    maxgch = max(glo + ghi for _, glo, ghi in groups)

    with tile.TileContext(nc) as tc:
        with tc.tile_pool(name="persist", bufs=1) as persist, \
             tc.tile_pool(name="xgp", bufs=3) as xgp, \
             tc.tile_pool(name="spool", bufs=12) as spool, \
             tc.tile_pool(name="outp", bufs=3) as outp, \
             tc.tile_pool(name="ps", bufs=4, space="PSUM") as ps, \
             tc.tile_pool(name="psd", bufs=2, space="PSUM") as psd:
            idx_t = persist.tile([P, F * 8], mybir.dt.int16)
            iota16_t = persist.tile([P, P], mybir.dt.float16)
            meta_t = persist.tile([P, 2 * F + BINS_PER_CORE], mybir.dt.float32)
            degrow_t = persist.tile([1, BINS_PER_CORE * P], mybir.dt.float16)
            biasrow_t = persist.tile([1, D], mybir.dt.float16)
            nc.sync.dma_start(out=idx_t[:], in_=idx_d[:, :])
            nc.sync.dma_start(out=iota16_t[:], in_=iota_d[:, :])
            nc.sync.dma_start(out=meta_t[:], in_=meta_d[:, :])
            nc.sync.dma_start(out=degrow_t[:], in_=degrow_d[:, :])
            nc.sync.dma_start(out=biasrow_t[:], in_=biasrow_d[:, :])
            iota_t = iota16_t[:]

            # pre-zero the gather tiles so padded slots never contain NaN/Inf
            for _w in range(3):
                wt = xgp.tile([P, maxgch * D], mybir.dt.float16, tag="xg")
                nc.vector.memset(wt[:], 0.0)

            _gq = [0]
            chunk_base = 0  # global chunk index across groups
            for pos, glo, ghi in groups:
                gch = glo + ghi
                xg = xgp.tile([P, gch * D], mybir.dt.float16, tag="xg")
                # gather calls: lo run then hi run, batched across bins
                calls = []  # (local chunk off, n chunks, is_high)
                for s in range(0, glo, GMAX):
                    calls.append((s, min(GMAX, glo - s), False))
                for s in range(0, ghi, GMAX):
                    calls.append((glo + s, min(GMAX, ghi - s), True))
                for s, n, hi in calls:
                    nc.gpsimd.dma_gather(
                        out_ap=xg[:, s * D : (s + n) * D].rearrange(
                            "p (k w) -> p k w", k=n),
                        in_ap=(x_d[SPLIT:N_NODES, :] if hi else x_d[0:SPLIT, :]),
                        idxs_ap=idx_t[:, (chunk_base + s) * 8
                                      : (chunk_base + s + n) * 8],
                        num_idxs=n * P,
                        num_idxs_reg=n * P,
                        elem_size=D,
                        queue_num=_gq[0] % NQ,
                    )
                    _gq[0] += 1
                # tiny PE reads of xg absorb the gather-DMA semaphore waits
                dummy = psd.tile([1, 1], mybir.dt.float32, tag="dummy")
                for s, n, hi in calls:
                    nc.tensor.matmul(out=dummy[:],
                                     lhsT=xg[:1, s * D : s * D + 1],
                                     rhs=xg[:1, s * D : s * D + 1],
                                     start=True, stop=True)
                # per-bin chunk lists: lo chunks at running lo offset, hi at
                # glo + running hi offset
                lo_off = 0
                hi_off = glo
                for p in pos:
                    psum = ps.tile([P, D], mybir.dt.float32, tag="psum")
                    nc.tensor.matmul(out=psum[:],
                                     lhsT=degrow_t[:, p * P : (p + 1) * P],
                                     rhs=biasrow_t[:, :],
                                     start=True, stop=False)
                    locs = ([lo_off + i for i in range(NLO[p])]
                            + [hi_off + i for i in range(NHI[p])])
                    lo_off += NLO[p]
                    hi_off += NHI[p]
                    for j, o in enumerate(locs):
                        q = chunk_base + o
                        S = spool.tile([P, P], mybir.dt.float16, tag="S")
                        nc.vector.tensor_scalar(
                            out=S[:], in0=iota_t,
                            scalar1=meta_t[:, q : q + 1],
                            scalar2=meta_t[:, F + q : F + q + 1],
                            op0=mybir.AluOpType.is_equal,
                            op1=mybir.AluOpType.mult,
                        )
                        nc.tensor.matmul(out=psum[:], lhsT=S[:],
                                         rhs=xg[:, o * D : (o + 1) * D],
                                         start=False, stop=(j == len(locs) - 1))
                    o_t = outp.tile([P, D], mybir.dt.float32, tag="o")
                    nc.scalar.activation(
                        out=o_t[:], in_=psum[:],
                        func=mybir.ActivationFunctionType.Copy,
                        scale=meta_t[:, 2 * F + p : 2 * F + p + 1])
                    nc.sync.dma_start(out=out_d[p * P : (p + 1) * P, :],
                                      in_=o_t[:])
                chunk_base += gch

    nc.compile()
    return nc


def _preprocess(x, edge_rows, edge_cols, adj_vals, bias):
    """Balanced bin assignment, col-sorted lo/hi bucketing, fp16 conversion,
    per-core device input arrays."""
    bin_id = (edge_rows // P).astype(np.int64)
    is_high = (edge_cols >= SPLIT).astype(np.int64)
    col64 = edge_cols.astype(np.int64)
    order = np.lexsort((col64, is_high, bin_id))
    b_s = bin_id[order]
    col_s = edge_cols[order].astype(np.int32)
    val_s = adj_vals[order].astype(np.float32)
    ri_s = (edge_rows[order] - b_s * P).astype(np.float32)

    n_tot = np.bincount(b_s, minlength=N_BINS)
    n_hi = np.bincount(b_s, weights=is_high[order].astype(np.float64),
                       minlength=N_BINS).astype(np.int64)
    n_lo = n_tot - n_hi
    starts = np.concatenate([[0], np.cumsum(n_tot)])[:N_BINS]

    # balanced assignment: sort bins by total edges; position p gets rank
    # group [8p, 8p+8), one bin per core
    rank = np.argsort(-n_tot, kind="stable")
    assign = rank.reshape(BINS_PER_CORE, N_CORES)  # [position, core] -> bin

    NLO = [max(1, int(max(_cdiv(int(n_lo[assign[p, c]]), P)
                          for c in range(N_CORES))))
           for p in range(BINS_PER_CORE)]
    NHI = [max(1, int(max(_cdiv(int(n_hi[assign[p, c]]), P)
                          for c in range(N_CORES))))
           for p in range(BINS_PER_CORE)]
    NCH = [NLO[p] + NHI[p] for p in range(BINS_PER_CORE)]
    F = sum(NCH)

    iota_np = np.tile(np.arange(P, dtype=np.float16), (P, 1))
    deg = np.bincount(edge_rows.astype(np.int64),
                      weights=adj_vals.astype(np.float64),
                      minlength=N_BINS * P).astype(np.float32)
    rdeg = np.ones(N_BINS * P, np.float32)
    nz = deg != 0
    rdeg[nz] = (1.0 / deg[nz]).astype(np.float32)
    deg[~nz] = 1.0

    x16 = np.ascontiguousarray(x, dtype=np.float16)
    bias16 = np.asarray(bias, np.float16).reshape(1, -1)

    # group chunk layout (identical across cores)
    n_groups = BINS_PER_CORE // GROUP

    in_maps = []
    for c in range(N_CORES):
        flat_cols = np.zeros(F * P, np.int32)   # gather index per slot
        ri_arr = np.zeros((P, F), np.float32)
        val_arr = np.zeros((P, F), np.float32)
        rdeg_arr = np.zeros((P, BINS_PER_CORE), np.float32)
        degrow = np.zeros(BINS_PER_CORE * P, np.float16)
        chunk_base = 0
        for g in range(n_groups):
            pos = range(GROUP * g, GROUP * (g + 1))
            # lo runs then hi runs
            lo_off = chunk_base
            hi_off = chunk_base + sum(NLO[p] for p in pos)
            for p in pos:
                bin_ = int(assign[p, c])
                s = int(starts[bin_])
                nl, nh = int(n_lo[bin_]), int(n_hi[bin_])
                # lo slots [lo_off*P, lo_off*P + NLO[p]*P)
                sl = slice(lo_off * P, lo_off * P + nl)
                flat_cols[sl.start : sl.start + nl] = col_s[s : s + nl]
                rr = ri_s[s : s + nl]
                vv = val_s[s : s + nl]
                loc = np.arange(nl)
                ri_arr[loc % P, lo_off + loc // P] = rr
                val_arr[loc % P, lo_off + loc // P] = vv
                # hi slots
                hs = hi_off * P
                flat_cols[hs : hs + nh] = col_s[s + nl : s + nl + nh] - SPLIT
                rr = ri_s[s + nl : s + nl + nh]
                vv = val_s[s + nl : s + nl + nh]
                loc = np.arange(nh)
                ri_arr[loc % P, hi_off + loc // P] = rr
                val_arr[loc % P, hi_off + loc // P] = vv
                rdeg_arr[:, p] = rdeg[bin_ * P : (bin_ + 1) * P]
                degrow[p * P : (p + 1) * P] = deg[bin_ * P : (bin_ + 1) * P
                                                  ].astype(np.float16)
                lo_off += NLO[p]
                hi_off += NHI[p]
            chunk_base = hi_off
        idx_np = np.tile(
            np.ascontiguousarray(flat_cols.astype(np.int16).reshape(-1, 16).T),
            (8, 1))
        meta_np = np.concatenate([ri_arr, val_arr, rdeg_arr],
                                 axis=1).astype(np.float32)
        in_maps.append({
            "x": x16,
            "idx": idx_np,
            "iota": iota_np,
            "meta": np.ascontiguousarray(meta_np),
            "degrow": degrow.reshape(1, -1),
            "biasrow": bias16,
        })
    return tuple(NLO), tuple(NHI), assign, in_maps


def _run(x, edge_rows, edge_cols, adj_vals, bias, trace=False, trace_cores=None):
    from concourse.bass_utils import run_bass_kernel_spmd

    NLO, NHI, assign, in_maps = _preprocess(x, edge_rows, edge_cols,
                                            adj_vals, bias)
    key = (NLO, NHI)
    if key not in _plan_cache:
        _plan_cache[key] = _build_program(list(NLO), list(NHI))
    nc = _plan_cache[key]
    kw = {}
    if trace:
        kw["trace"] = True
        if trace_cores is not None:
            kw["trace_cores"] = trace_cores
    res = run_bass_kernel_spmd(nc, in_maps, core_ids=list(range(N_CORES)), **kw)
    # unscramble: core c position p holds bin assign[p, c]
    full = np.empty((N_BINS * P, D), np.float32)
    for c in range(N_CORES):
        oc = res.results[c]["out"]
        for p in range(BINS_PER_CORE):
            b = int(assign[p, c])
            full[b * P : (b + 1) * P] = oc[p * P : (p + 1) * P]
    return full[:N_NODES].astype(np.float32), res


def kernel(x, edge_rows, edge_cols, adj_vals, bias):
    out, _ = _run(np.asarray(x), np.asarray(edge_rows), np.asarray(edge_cols),
                  np.asarray(adj_vals), np.asarray(bias))
    return out


# revision 20
# speedup vs baseline: 1.3934x; 1.0709x over previous
"""GNN message-passing (SpMM + mean-normalize + bias) Trainium2 kernel.

out[r] = (sum_{e: rows[e]==r} vals[e] * x[cols[e]]) / deg[r] + bias,
deg[r] = sum vals[e], rows with deg==0 -> bias.

Strategy (8 NeuronCores, SPMD):
  - Pad N=40000 rows to 40960 = 320 bins x 128 rows. Bins are assigned to
    (core, position) pairs in balanced order (sorted by edge count, rank
    groups of 8 across cores) so the SPMD-shared chunk schedule has minimal
    padding. Host unscrambles the output rows.
  - Per bin, edges are split into a low group (col < 32768) and a high
    group (col >= 32768), each padded to a multiple of 128 with null
    edges (val=0), because dma_gather carries int16 indices. Edges are
    sorted by col inside each group for DMA locality.
  - x is converted to fp16 on the host. Gathers fetch 256B rows; bins are
    processed in groups of 5 sharing one xg tile so gather calls batch up
    to 1024 indices regardless of bin boundaries.
  - Device per chunk of 128 edges: a one-hot selection matrix
    S[t,r] = (ri[t]==r)*val[t] (fp16) is built on the vector engine from an
    iota tile, then the tensor engine computes psum[r,f] += S^T @ xg
    (PSUM fp32 accumulation). The accumulation is seeded with
    deg[r]*bias[f] via a tiny fp16 matmul, so the epilogue is just
    out = psum * rdeg on the scalar engine (deg==0 rows have rdeg=1,
    deg=1 -> out=bias).
"""
import sys

sys.path.insert(0, "/opt/trn_rl_repo")

import numpy as np

N_NODES = 40000
N_EDGES = 640000
D = 128
P = 128
N_CORES = 8
BINS_PER_CORE = 40
N_BINS = N_CORES * BINS_PER_CORE          # 320 (rows padded to 40960)
SPLIT = 32768                             # int16-safe index split
GROUP = 5                                 # bins per gather group
GMAX = 8                                  # chunks (1024 idx) per dma_gather

_plan_cache: dict = {}


def _patch_ldw_opt():
    """Enable walrus's LDW dedup pass (second matmul on the same stationary
    S skips its LoadWeights)."""
    import concourse.bass_utils as bu

    if getattr(bu, "_ldw_patched", False):
        return
    orig = bu.run_command

    def patched(argv, **kw):
        argv = ["--enable-ldw-opt=true" if a == "--enable-ldw-opt=false" else a
                for a in argv]
        return orig(argv, **kw)

    bu.run_command = patched
    bu._ldw_patched = True


def _cdiv(a, b):
    return -(-a // b)


SWIN = 8  # chunks per wide S-build custom-DVE op


def _register_sbuild_op():
    """Register SPMM_ONEHOT_ANT: out[p,k] = (k == in1[p,k]) * in0[p,k].

    With in1 = (ri + 128*(chunk%SWIN)) broadcast along the last (stride-0)
    dim and in0 = val broadcast likewise, one op builds SWIN one-hot
    S tiles [128, SWIN*128] in a single DVE pass.
    """
    import re

    import concourse.dve_ops as do
    from concourse.dve_spec import Spec, Src0, Src1, Zero, Idx, eq, select

    if hasattr(do, "SPMM_ONEHOT_ANT"):
        return do.SPMM_ONEHOT_ANT

    def _ref(in0, in1, s0, s1, imm2):
        a = np.asarray(in0, np.float32)
        b = np.asarray(in1, np.float32)
        p = a.shape[0]
        af = a.reshape(p, -1)
        bf = b.reshape(p, -1)
        k = np.arange(af.shape[1], dtype=np.float32)[None, :]
        return np.where(bf == k, af, 0.0).reshape(a.shape)

    spec = Spec(body=select(eq(Idx, Src1), Src0, Zero), reference=_ref)

    def _make(sha):
        return do.DveOp("SPMM_ONEHOT_ANT", spec, subdim=False,
                        uops_sha=dict(sha))

    # opcode row must exist before compile() (it bakes the opcode)
    do._SUB_OPCODE_FOR_NAME["SPMM_ONEHOT_ANT"] = (
        do._CUSTOM_DVE_ROW_BASE + len(do.OPS))
    from concourse.dve_table_gen import dve_ver_for

    ver = dve_ver_for("TRN2")
    op = _make({})
    try:
        op.compile(ver)
    except ValueError as e:
        m = re.search(r'uops_sha\["(v\d)"\]="([0-9a-f]+)"', str(e))
        assert m, f"cannot parse uops sha from: {e}"
        op = _make({m.group(1): m.group(2)})
    do.OPS.append(op)
    do.CUSTOM_DVE_SPECS[op.name] = op.spec
    do.SPMM_ONEHOT_ANT = op
    return op


def _build_program(NLO, NHI):
    """Build+compile the SPMD Bass program for the given per-position chunk
    schedule (shared by all cores)."""
    import concourse.bacc as bacc
    import concourse.tile as tile
    from concourse import mybir

    sbuild_op = _register_sbuild_op()

    NCH = [NLO[p] + NHI[p] for p in range(BINS_PER_CORE)]
    F = sum(NCH)

    NQ = 4
    nc = bacc.Bacc(num_swdge_queues=NQ)
    x_d = nc.dram_tensor("x", [N_NODES, D], mybir.dt.float16,
                         kind="ExternalInput")
    idx_d = nc.dram_tensor("idx", [P, F * 8], mybir.dt.int16,
                           kind="ExternalInput")
    meta_d = nc.dram_tensor("meta", [P, 2 * F], mybir.dt.float16,
                            kind="ExternalInput")
    onesrow_d = nc.dram_tensor("onesrow", [1, P], mybir.dt.float16,
                               kind="ExternalInput")
    biasrow_d = nc.dram_tensor("biasrow", [1, D], mybir.dt.float16,
                               kind="ExternalInput")
    out_d = nc.dram_tensor("out", [BINS_PER_CORE * P, D], mybir.dt.float32,
                           kind="ExternalOutput")

    # group schedule: positions [5g, 5g+5); chunk layout per group:
    # [lo p0][lo p1]..[lo p4][hi p0]..[hi p4]
    groups = []
    for g in range(BINS_PER_CORE // GROUP):
        pos = list(range(GROUP * g, GROUP * (g + 1)))
        glo = sum(NLO[p] for p in pos)
        ghi = sum(NHI[p] for p in pos)
        groups.append((pos, glo, ghi))
    maxgch = max(glo + ghi for _, glo, ghi in groups)

    with tile.TileContext(nc) as tc:
        with tc.tile_pool(name="persist", bufs=1) as persist, \
             tc.tile_pool(name="xgp", bufs=3) as xgp, \
             tc.tile_pool(name="spool", bufs=6) as spool, \
             tc.tile_pool(name="outp", bufs=3) as outp, \
             tc.tile_pool(name="ps", bufs=4, space="PSUM") as ps, \
             tc.tile_pool(name="psd", bufs=2, space="PSUM") as psd:
            idx_t = persist.tile([P, F * 8], mybir.dt.int16)
            meta_t = persist.tile([P, 2 * F], mybir.dt.float16)
            onesrow_t = persist.tile([1, P], mybir.dt.float16)
            biasrow_t = persist.tile([1, D], mybir.dt.float16)
            nc.sync.dma_start(out=idx_t[:], in_=idx_d[:, :])
            nc.sync.dma_start(out=meta_t[:], in_=meta_d[:, :])
            nc.sync.dma_start(out=onesrow_t[:], in_=onesrow_d[:, :])
            nc.sync.dma_start(out=biasrow_t[:], in_=biasrow_d[:, :])

            swin_tiles = {}

            def get_S(w):
                if w not in swin_tiles:
                    wc = min(SWIN, F - SWIN * w)
                    s8 = spool.tile([P, wc * P], mybir.dt.float16, tag="S8")
                    val_ap = meta_t[:, F + SWIN * w : F + SWIN * w + wc
                                    ].unsqueeze(2).broadcast_to([P, wc, P])
                    ri_ap = meta_t[:, SWIN * w : SWIN * w + wc
                                   ].unsqueeze(2).broadcast_to([P, wc, P])
                    nc.vector._custom_dve(sbuild_op, out=s8[:],
                                          in0=val_ap, in1=ri_ap)
                    swin_tiles[w] = s8
                return swin_tiles[w]

            # pre-zero the gather tiles so padded slots never contain NaN/Inf
            for _w in range(3):
                wt = xgp.tile([P, maxgch * D], mybir.dt.float16, tag="xg")
                nc.vector.memset(wt[:], 0.0)

            _gq = [0]
            chunk_base = 0  # global chunk index across groups
            for pos, glo, ghi in groups:
                gch = glo + ghi
                xg = xgp.tile([P, gch * D], mybir.dt.float16, tag="xg")
                # gather calls: lo run then hi run, batched across bins
                calls = []  # (local chunk off, n chunks, is_high)
                for s in range(0, glo, GMAX):
                    calls.append((s, min(GMAX, glo - s), False))
                for s in range(0, ghi, GMAX):
                    calls.append((glo + s, min(GMAX, ghi - s), True))
                for s, n, hi in calls:
                    nc.gpsimd.dma_gather(
                        out_ap=xg[:, s * D : (s + n) * D].rearrange(
                            "p (k w) -> p k w", k=n),
                        in_ap=(x_d[SPLIT:N_NODES, :] if hi else x_d[0:SPLIT, :]),
                        idxs_ap=idx_t[:, (chunk_base + s) * 8
                                      : (chunk_base + s + n) * 8],
                        num_idxs=n * P,
                        num_idxs_reg=n * P,
                        elem_size=D,
                        queue_num=_gq[0] % NQ,
                    )
                    _gq[0] += 1
                # tiny PE reads of xg absorb the gather-DMA semaphore waits
                dummy = psd.tile([1, 1], mybir.dt.float32, tag="dummy")
                for s, n, hi in calls:
                    nc.tensor.matmul(out=dummy[:],
                                     lhsT=xg[:1, s * D : s * D + 1],
                                     rhs=xg[:1, s * D : s * D + 1],
                                     start=True, stop=True)
                # per-bin chunk lists: lo chunks at running lo offset, hi at
                # glo + running hi offset
                lo_off = 0
                hi_off = glo
                for p in pos:
                    psum = ps.tile([P, D], mybir.dt.float32, tag="psum")
                    nc.tensor.matmul(out=psum[:],
                                     lhsT=onesrow_t[:, :],
                                     rhs=biasrow_t[:, :],
                                     start=True, stop=False)
                    locs = ([lo_off + i for i in range(NLO[p])]
                            + [hi_off + i for i in range(NHI[p])])
                    lo_off += NLO[p]
                    hi_off += NHI[p]
                    for j, o in enumerate(locs):
                        q = chunk_base + o
                        s8 = get_S(q // SWIN)
                        c = q % SWIN
                        nc.tensor.matmul(out=psum[:],
                                         lhsT=s8[:, c * P : (c + 1) * P],
                                         rhs=xg[:, o * D : (o + 1) * D],
                                         start=False, stop=(j == len(locs) - 1))
                    o_t = outp.tile([P, D], mybir.dt.float32, tag="o")
                    nc.scalar.activation(
                        out=o_t[:], in_=psum[:],
                        func=mybir.ActivationFunctionType.Copy,
                        scale=1.0)
                    nc.sync.dma_start(out=out_d[p * P : (p + 1) * P, :],
                                      in_=o_t[:])
                chunk_base += gch

    nc.compile()
    return nc


def _preprocess(x, edge_rows, edge_cols, adj_vals, bias):
    """Balanced bin assignment, col-sorted lo/hi bucketing, fp16 conversion,
    per-core device input arrays."""
    bin_id = (edge_rows // P).astype(np.int64)
    is_high = (edge_cols >= SPLIT).astype(np.int64)
    col64 = edge_cols.astype(np.int64)
    order = np.lexsort((col64, is_high, bin_id))
    b_s = bin_id[order]
    col_s = edge_cols[order].astype(np.int32)
    ri_s = (edge_rows[order] - b_s * P).astype(np.float32)

    # fold 1/deg into the edge values so the device epilogue is a plain copy
    deg_full = np.bincount(edge_rows.astype(np.int64),
                           weights=adj_vals.astype(np.float64),
                           minlength=N_BINS * P)
    rdeg_full = np.ones(N_BINS * P)
    nzf = deg_full != 0
    rdeg_full[nzf] = 1.0 / deg_full[nzf]
    val_s = (adj_vals[order].astype(np.float64)
             * rdeg_full[edge_rows[order]]).astype(np.float32)

    n_tot = np.bincount(b_s, minlength=N_BINS)
    n_hi = np.bincount(b_s, weights=is_high[order].astype(np.float64),
                       minlength=N_BINS).astype(np.int64)
    n_lo = n_tot - n_hi
    starts = np.concatenate([[0], np.cumsum(n_tot)])[:N_BINS]

    # balanced assignment: sort bins by total edges; position p gets rank
    # group [8p, 8p+8), one bin per core
    rank = np.argsort(-n_tot, kind="stable")
    assign = rank.reshape(BINS_PER_CORE, N_CORES)  # [position, core] -> bin

    NLO = [max(1, int(max(_cdiv(int(n_lo[assign[p, c]]), P)
                          for c in range(N_CORES))))
           for p in range(BINS_PER_CORE)]
    NHI = [max(1, int(max(_cdiv(int(n_hi[assign[p, c]]), P)
                          for c in range(N_CORES))))
           for p in range(BINS_PER_CORE)]
    NCH = [NLO[p] + NHI[p] for p in range(BINS_PER_CORE)]
    F = sum(NCH)

    x16 = np.ascontiguousarray(x, dtype=np.float16)
    bias16 = np.asarray(bias, np.float16).reshape(1, -1)

    # group chunk layout (identical across cores)
    n_groups = BINS_PER_CORE // GROUP

    in_maps = []
    for c in range(N_CORES):
        flat_cols = np.zeros(F * P, np.int32)   # gather index per slot
        ri_arr = np.zeros((P, F), np.float32)
        val_arr = np.zeros((P, F), np.float32)
        chunk_base = 0
        for g in range(n_groups):
            pos = range(GROUP * g, GROUP * (g + 1))
            # lo runs then hi runs
            lo_off = chunk_base
            hi_off = chunk_base + sum(NLO[p] for p in pos)
            for p in pos:
                bin_ = int(assign[p, c])
                s = int(starts[bin_])
                nl, nh = int(n_lo[bin_]), int(n_hi[bin_])
                # lo slots [lo_off*P, lo_off*P + NLO[p]*P)
                sl = slice(lo_off * P, lo_off * P + nl)
                flat_cols[sl.start : sl.start + nl] = col_s[s : s + nl]
                rr = ri_s[s : s + nl]
                vv = val_s[s : s + nl]
                loc = np.arange(nl)
                ri_arr[loc % P, lo_off + loc // P] = rr
                val_arr[loc % P, lo_off + loc // P] = vv
                # hi slots
                hs = hi_off * P
                flat_cols[hs : hs + nh] = col_s[s + nl : s + nl + nh] - SPLIT
                rr = ri_s[s + nl : s + nl + nh]
                vv = val_s[s + nl : s + nl + nh]
                loc = np.arange(nh)
                ri_arr[loc % P, hi_off + loc // P] = rr
                val_arr[loc % P, hi_off + loc // P] = vv
                lo_off += NLO[p]
                hi_off += NHI[p]
            chunk_base = hi_off
        idx_np = np.tile(
            np.ascontiguousarray(flat_cols.astype(np.int16).reshape(-1, 16).T),
            (8, 1))
        # shift ri by 128*(chunk index within its SWIN window): the wide
        # S-build op compares against the global free index
        ri_arr += (np.arange(F, dtype=np.float32) % SWIN)[None, :] * P
        meta_np = np.concatenate([ri_arr, val_arr], axis=1).astype(np.float16)
        in_maps.append({
            "x": x16,
            "idx": idx_np,
            "meta": np.ascontiguousarray(meta_np),
            "onesrow": np.ones((1, P), np.float16),
            "biasrow": bias16,
        })
    return tuple(NLO), tuple(NHI), assign, in_maps


def _run(x, edge_rows, edge_cols, adj_vals, bias, trace=False, trace_cores=None):
    from concourse.bass_utils import run_bass_kernel_spmd

    NLO, NHI, assign, in_maps = _preprocess(x, edge_rows, edge_cols,
                                            adj_vals, bias)
    key = (NLO, NHI)
    if key not in _plan_cache:
        _plan_cache[key] = _build_program(list(NLO), list(NHI))
    nc = _plan_cache[key]
    kw = {}
    if trace:
        kw["trace"] = True
        if trace_cores is not None:
            kw["trace_cores"] = trace_cores
    res = run_bass_kernel_spmd(nc, in_maps, core_ids=list(range(N_CORES)), **kw)
    # unscramble: core c position p holds bin assign[p, c]
    full = np.empty((N_BINS * P, D), np.float32)
    for c in range(N_CORES):
        oc = res.results[c]["out"]
        for p in range(BINS_PER_CORE):
            b = int(assign[p, c])
            full[b * P : (b + 1) * P] = oc[p * P : (p + 1) * P]
    return full[:N_NODES].astype(np.float32), res


def kernel(x, edge_rows, edge_cols, adj_vals, bias):
    out, _ = _run(np.asarray(x), np.asarray(edge_rows), np.asarray(edge_cols),
                  np.asarray(adj_vals), np.asarray(bias))
    return out


# revision 23
# speedup vs baseline: 1.7813x; 1.2784x over previous
"""GNN message-passing (SpMM + mean-normalize + bias) Trainium2 kernel.

out[r] = (sum_{e: rows[e]==r} vals[e] * x[cols[e]]) / deg[r] + bias,
deg[r] = sum vals[e], rows with deg==0 -> bias.

Strategy (8 NeuronCores, SPMD):
  - Pad N=40000 rows to 40960 = 320 bins x 128 rows. Bins are assigned to
    (core, position) pairs in balanced order (sorted by edge count, rank
    groups of 8 across cores) so the SPMD-shared chunk schedule has minimal
    padding. Host unscrambles the output rows.
  - Per bin, edges are split into a low group (col < 32768) and a high
    group (col >= 32768), each padded to a multiple of 128 with null
    edges (val=0), because dma_gather carries int16 indices. Edges are
    sorted by col inside each group for DMA locality.
  - x is converted to fp16 on the host. Gathers fetch 256B rows; bins are
    processed in groups of 5 sharing one xg tile so gather calls batch up
    to 1024 indices regardless of bin boundaries.
  - Device per chunk of 128 edges: a one-hot selection matrix
    S[t,r] = (ri[t]==r)*val[t] (fp16) is built on the vector engine from an
    iota tile, then the tensor engine computes psum[r,f] += S^T @ xg
    (PSUM fp32 accumulation). The accumulation is seeded with
    deg[r]*bias[f] via a tiny fp16 matmul, so the epilogue is just
    out = psum * rdeg on the scalar engine (deg==0 rows have rdeg=1,
    deg=1 -> out=bias).
"""
import sys

sys.path.insert(0, "/opt/trn_rl_repo")

import numpy as np

N_NODES = 40000
N_EDGES = 640000
D = 128
P = 128
N_CORES = 8
BINS_PER_CORE = 40
N_BINS = N_CORES * BINS_PER_CORE          # 320 (rows padded to 40960)
SPLIT = 32768                             # int16-safe index split
GROUP = 5                                 # bins per gather group
GMAX = 8                                  # chunks (1024 idx) per dma_gather

_plan_cache: dict = {}


def _cdiv(a, b):
    return -(-a // b)


SWIN = 8  # chunks per wide S-build custom-DVE op


def _register_sbuild_op():
    """Register SPMM_ONEHOT_ANT: out[p,k] = (k == in1[p,k]) * in0[p,k].

    With in1 = (ri + 128*(chunk%SWIN)) broadcast along the last (stride-0)
    dim and in0 = val broadcast likewise, one op builds SWIN one-hot
    S tiles [128, SWIN*128] in a single DVE pass.
    """
    import re

    import concourse.dve_ops as do
    from concourse.dve_spec import Spec, Src0, Src1, Zero, Idx, eq, select

    if hasattr(do, "SPMM_ONEHOT_ANT"):
        return do.SPMM_ONEHOT_ANT

    def _ref(in0, in1, s0, s1, imm2):
        a = np.asarray(in0, np.float32)
        b = np.asarray(in1, np.float32)
        p = a.shape[0]
        af = a.reshape(p, -1)
        bf = b.reshape(p, -1)
        k = np.arange(af.shape[1], dtype=np.float32)[None, :]
        return np.where(bf == k, af, 0.0).reshape(a.shape)

    spec = Spec(body=select(eq(Idx, Src1), Src0, Zero), reference=_ref)

    def _make(sha):
        return do.DveOp("SPMM_ONEHOT_ANT", spec, subdim=False,
                        uops_sha=dict(sha))

    # opcode row must exist before compile() (it bakes the opcode)
    do._SUB_OPCODE_FOR_NAME["SPMM_ONEHOT_ANT"] = (
        do._CUSTOM_DVE_ROW_BASE + len(do.OPS))
    from concourse.dve_table_gen import dve_ver_for

    ver = dve_ver_for("TRN2")
    op = _make({})
    try:
        op.compile(ver)
    except ValueError as e:
        m = re.search(r'uops_sha\["(v\d)"\]="([0-9a-f]+)"', str(e))
        assert m, f"cannot parse uops sha from: {e}"
        op = _make({m.group(1): m.group(2)})
    do.OPS.append(op)
    do.CUSTOM_DVE_SPECS[op.name] = op.spec
    do.SPMM_ONEHOT_ANT = op
    return op


def _build_program(NLO, NHI):
    """Build+compile the SPMD Bass program for the given per-position chunk
    schedule (shared by all cores)."""
    import concourse.bacc as bacc
    import concourse.tile as tile
    from concourse import mybir

    sbuild_op = _register_sbuild_op()

    NCH = [NLO[p] + NHI[p] for p in range(BINS_PER_CORE)]
    F = sum(NCH)

    NQ = 4
    nc = bacc.Bacc(num_swdge_queues=NQ)
    x_d = nc.dram_tensor("x", [N_NODES, D], mybir.dt.float16,
                         kind="ExternalInput")
    idx_d = nc.dram_tensor("idx", [P, F * 8], mybir.dt.int16,
                           kind="ExternalInput")
    meta_d = nc.dram_tensor("meta", [P, 2 * F], mybir.dt.float16,
                            kind="ExternalInput")
    onesrow_d = nc.dram_tensor("onesrow", [1, P], mybir.dt.float16,
                               kind="ExternalInput")
    biasrow_d = nc.dram_tensor("biasrow", [1, D], mybir.dt.float16,
                               kind="ExternalInput")
    out_d = nc.dram_tensor("out", [BINS_PER_CORE * P, D], mybir.dt.float32,
                           kind="ExternalOutput")

    # group schedule: positions [5g, 5g+5); chunk layout per group:
    # [lo p0][lo p1]..[lo p4][hi p0]..[hi p4]
    groups = []
    for g in range(BINS_PER_CORE // GROUP):
        pos = list(range(GROUP * g, GROUP * (g + 1)))
        glo = sum(NLO[p] for p in pos)
        ghi = sum(NHI[p] for p in pos)
        groups.append((pos, glo, ghi))
    maxgch = max(glo + ghi for _, glo, ghi in groups)

    with tile.TileContext(nc) as tc:
        with tc.tile_pool(name="persist", bufs=1) as persist, \
             tc.tile_pool(name="xgp", bufs=5) as xgp, \
             tc.tile_pool(name="spool", bufs=6) as spool, \
             tc.tile_pool(name="outp", bufs=3) as outp, \
             tc.tile_pool(name="ps", bufs=4, space="PSUM") as ps, \
             tc.tile_pool(name="psd", bufs=2, space="PSUM") as psd:
            idx_t = persist.tile([P, F * 8], mybir.dt.int16)
            meta_t = persist.tile([P, 2 * F], mybir.dt.float16)
            onesrow_t = persist.tile([1, P], mybir.dt.float16)
            biasrow_t = persist.tile([1, D], mybir.dt.float16)
            nc.sync.dma_start(out=idx_t[:], in_=idx_d[:, :])
            nc.sync.dma_start(out=meta_t[:], in_=meta_d[:, :])
            nc.sync.dma_start(out=onesrow_t[:], in_=onesrow_d[:, :])
            nc.sync.dma_start(out=biasrow_t[:], in_=biasrow_d[:, :])

            swin_tiles = {}

            def get_S(w):
                if w not in swin_tiles:
                    wc = min(SWIN, F - SWIN * w)
                    s8 = spool.tile([P, wc * P], mybir.dt.float16, tag="S8")
                    val_ap = meta_t[:, F + SWIN * w : F + SWIN * w + wc
                                    ].unsqueeze(2).broadcast_to([P, wc, P])
                    ri_ap = meta_t[:, SWIN * w : SWIN * w + wc
                                   ].unsqueeze(2).broadcast_to([P, wc, P])
                    nc.vector._custom_dve(sbuild_op, out=s8[:],
                                          in0=val_ap, in1=ri_ap)
                    swin_tiles[w] = s8
                return swin_tiles[w]

            # no xg pre-zero needed: every slot (incl. padding, idx 0) is
            # written by the gathers, and S carries val=0 for pad slots
            _gq = [0]
            chunk_base = 0  # global chunk index across groups
            for pos, glo, ghi in groups:
                gch = glo + ghi
                xg = xgp.tile([P, gch * D], mybir.dt.float16, tag="xg")
                # gather calls: lo run then hi run, batched across bins
                calls = []  # (local chunk off, n chunks, is_high)
                for s in range(0, glo, GMAX):
                    calls.append((s, min(GMAX, glo - s), False))
                for s in range(0, ghi, GMAX):
                    calls.append((glo + s, min(GMAX, ghi - s), True))
                for s, n, hi in calls:
                    nc.gpsimd.dma_gather(
                        out_ap=xg[:, s * D : (s + n) * D].rearrange(
                            "p (k w) -> p k w", k=n),
                        in_ap=(x_d[SPLIT:N_NODES, :] if hi else x_d[0:SPLIT, :]),
                        idxs_ap=idx_t[:, (chunk_base + s) * 8
                                      : (chunk_base + s + n) * 8],
                        num_idxs=n * P,
                        num_idxs_reg=n * P,
                        elem_size=D,
                        queue_num=_gq[0] % NQ,
                    )
                    _gq[0] += 1
                # tiny PE reads of xg absorb the gather-DMA semaphore waits
                dummy = psd.tile([1, 1], mybir.dt.float32, tag="dummy")
                for s, n, hi in calls:
                    nc.tensor.matmul(out=dummy[:],
                                     lhsT=xg[:1, s * D : s * D + 1],
                                     rhs=xg[:1, s * D : s * D + 1],
                                     start=True, stop=True)
                # per-bin chunk lists: lo chunks at running lo offset, hi at
                # glo + running hi offset
                lo_off = 0
                hi_off = glo
                for p in pos:
                    psum = ps.tile([P, D], mybir.dt.float32, tag="psum")
                    nc.tensor.matmul(out=psum[:],
                                     lhsT=onesrow_t[:, :],
                                     rhs=biasrow_t[:, :],
                                     start=True, stop=False)
                    locs = ([lo_off + i for i in range(NLO[p])]
                            + [hi_off + i for i in range(NHI[p])])
                    lo_off += NLO[p]
                    hi_off += NHI[p]
                    for j, o in enumerate(locs):
                        q = chunk_base + o
                        s8 = get_S(q // SWIN)
                        c = q % SWIN
                        nc.tensor.matmul(out=psum[:],
                                         lhsT=s8[:, c * P : (c + 1) * P],
                                         rhs=xg[:, o * D : (o + 1) * D],
                                         start=False, stop=(j == len(locs) - 1))
                    o_t = outp.tile([P, D], mybir.dt.float32, tag="o")
                    nc.scalar.activation(
                        out=o_t[:], in_=psum[:],
                        func=mybir.ActivationFunctionType.Copy,
                        scale=1.0)
                    nc.sync.dma_start(out=out_d[p * P : (p + 1) * P, :],
                                      in_=o_t[:])
                chunk_base += gch

    nc.compile()
    return nc


def _preprocess(x, edge_rows, edge_cols, adj_vals, bias):
    """Balanced bin assignment, col-sorted lo/hi bucketing, fp16 conversion,
    per-core device input arrays."""
    bin_id = (edge_rows // P).astype(np.int64)
    is_high = (edge_cols >= SPLIT).astype(np.int64)
    col64 = edge_cols.astype(np.int64)
    order = np.lexsort((col64, is_high, bin_id))
    b_s = bin_id[order]
    col_s = edge_cols[order].astype(np.int32)
    ri_s = (edge_rows[order] - b_s * P).astype(np.float32)

    # fold 1/deg into the edge values so the device epilogue is a plain copy
    deg_full = np.bincount(edge_rows.astype(np.int64),
                           weights=adj_vals.astype(np.float64),
                           minlength=N_BINS * P)
    rdeg_full = np.ones(N_BINS * P)
    nzf = deg_full != 0
    rdeg_full[nzf] = 1.0 / deg_full[nzf]
    val_s = (adj_vals[order].astype(np.float64)
             * rdeg_full[edge_rows[order]]).astype(np.float32)

    n_tot = np.bincount(b_s, minlength=N_BINS)
    n_hi = np.bincount(b_s, weights=is_high[order].astype(np.float64),
                       minlength=N_BINS).astype(np.int64)
    n_lo = n_tot - n_hi
    starts = np.concatenate([[0], np.cumsum(n_tot)])[:N_BINS]

    # balanced assignment: sort bins by total edges; position p gets rank
    # group [8p, 8p+8), one bin per core
    rank = np.argsort(-n_tot, kind="stable")
    assign = rank.reshape(BINS_PER_CORE, N_CORES)  # [position, core] -> bin

    NLO = [max(1, int(max(_cdiv(int(n_lo[assign[p, c]]), P)
                          for c in range(N_CORES))))
           for p in range(BINS_PER_CORE)]
    NHI = [max(1, int(max(_cdiv(int(n_hi[assign[p, c]]), P)
                          for c in range(N_CORES))))
           for p in range(BINS_PER_CORE)]
    NCH = [NLO[p] + NHI[p] for p in range(BINS_PER_CORE)]
    F = sum(NCH)

    x16 = np.ascontiguousarray(x, dtype=np.float16)
    bias16 = np.asarray(bias, np.float16).reshape(1, -1)

    # group chunk layout (identical across cores)
    n_groups = BINS_PER_CORE // GROUP

    in_maps = []
    for c in range(N_CORES):
        flat_cols = np.zeros(F * P, np.int32)   # gather index per slot
        ri_arr = np.zeros((P, F), np.float32)
        val_arr = np.zeros((P, F), np.float32)
        chunk_base = 0
        for g in range(n_groups):
            pos = range(GROUP * g, GROUP * (g + 1))
            # lo runs then hi runs
            lo_off = chunk_base
            hi_off = chunk_base + sum(NLO[p] for p in pos)
            for p in pos:
                bin_ = int(assign[p, c])
                s = int(starts[bin_])
                nl, nh = int(n_lo[bin_]), int(n_hi[bin_])
                # lo slots [lo_off*P, lo_off*P + NLO[p]*P)
                sl = slice(lo_off * P, lo_off * P + nl)
                flat_cols[sl.start : sl.start + nl] = col_s[s : s + nl]
                rr = ri_s[s : s + nl]
                vv = val_s[s : s + nl]
                loc = np.arange(nl)
                ri_arr[loc % P, lo_off + loc // P] = rr
                val_arr[loc % P, lo_off + loc // P] = vv
                # hi slots
                hs = hi_off * P
                flat_cols[hs : hs + nh] = col_s[s + nl : s + nl + nh] - SPLIT
                rr = ri_s[s + nl : s + nl + nh]
                vv = val_s[s + nl : s + nl + nh]
                loc = np.arange(nh)
                ri_arr[loc % P, hi_off + loc // P] = rr
                val_arr[loc % P, hi_off + loc // P] = vv
                lo_off += NLO[p]
                hi_off += NHI[p]
            chunk_base = hi_off
        idx_np = np.tile(
            np.ascontiguousarray(flat_cols.astype(np.int16).reshape(-1, 16).T),
            (8, 1))
        # shift ri by 128*(chunk index within its SWIN window): the wide
        # S-build op compares against the global free index
        ri_arr += (np.arange(F, dtype=np.float32) % SWIN)[None, :] * P
        meta_np = np.concatenate([ri_arr, val_arr], axis=1).astype(np.float16)
        in_maps.append({
            "x": x16,
            "idx": idx_np,
            "meta": np.ascontiguousarray(meta_np),
            "onesrow": np.ones((1, P), np.float16),
            "biasrow": bias16,
        })
    return tuple(NLO), tuple(NHI), assign, in_maps


def _run(x, edge_rows, edge_cols, adj_vals, bias, trace=False, trace_cores=None):
    from concourse.bass_utils import run_bass_kernel_spmd

    NLO, NHI, assign, in_maps = _preprocess(x, edge_rows, edge_cols,
                                            adj_vals, bias)
    key = (NLO, NHI)
    if key not in _plan_cache:
        _plan_cache[key] = _build_program(list(NLO), list(NHI))
    nc = _plan_cache[key]
    kw = {}
    if trace:
        kw["trace"] = True
        if trace_cores is not None:
            kw["trace_cores"] = trace_cores
    res = run_bass_kernel_spmd(nc, in_maps, core_ids=list(range(N_CORES)), **kw)
    # unscramble: core c position p holds bin assign[p, c]
    full = np.empty((N_BINS * P, D), np.float32)
    for c in range(N_CORES):
        oc = res.results[c]["out"]
        for p in range(BINS_PER_CORE):
            b = int(assign[p, c])
            full[b * P : (b + 1) * P] = oc[p * P : (p + 1) * P]
    return full[:N_NODES].astype(np.float32), res


def kernel(x, edge_rows, edge_cols, adj_vals, bias):
    out, _ = _run(np.asarray(x), np.asarray(edge_rows), np.asarray(edge_cols),
                  np.asarray(adj_vals), np.asarray(bias))
    return out
